# revision 10
# baseline (speedup 1.0000x reference)
"""Trainium2 Bass kernel for MessagePassingWithPhase.

Reference computation (B=2, N=512, D=128, O=4):
    recv = X @ W1r ; send = X @ W1s
    hidden[b,i,j,:]  = relu(recv[b,i] + send[b,j] + b1)
    messages         = hidden @ W2 + b2
    gate             = sigmoid(cos(phi_i - phi_j) @ Wg + bg)
    agg[b,i]         = sum_j mask[i,j] * (messages * gate)[b,i,j] / cnt_i
    out              = X + (relu(X@Wu1x + agg@Wu1a + bu1) @ Wu2 + bu2)

Device mapping (8 cores, receiver axis sharded, 64 receivers/core, both batches):
  * All tiles feature-major: (D=128 partitions, node index on free axis).
  * cos(phi_i-phi_j) = cos(phi_i)cos(phi_j) + sin(phi_i)sin(phi_j): the gate
    linear becomes a K=9 matmul  Wg3.T @ r9  with
      r9[0:8,j] = P8[o,j] * P8[o,i]   (P8 = [cos(phi); sin(phi)], host-computed)
      r9[8,j]   = -50 * (1 - mask[i,j])   and Wg3 = [[Wg;Wg]; ones]
    so sigmoid(..) == gate masked to ~0 on non-neighbors: the masked mean
    becomes a plain sum over j scaled by 1/cnt_i.
  * Per receiver i:  H = relu(sendT + recvb[:,i])          (one tensor_scalar)
                     M = W2.T @ H            (PSUM)        (one matmul)
                     Glin = Wg3.T @ r9_i     (PSUM)        (one matmul)
                     G = sigmoid(Glin + bg)                (ACT, batched pairs)
                     agg[:,i] = sum_j (M + b2) * G         (one fused
                        scalar_tensor_tensor with accum_out reduction)
"""

import os
import sys
import numpy as np

for _p in ("/opt/trn_rl_repo", "/root/.axon_site/_ro/trn_rl_repo"):
    if os.path.isdir(_p) and _p not in sys.path:
        sys.path.append(_p)

B, N, D, O = 2, 512, 128, 4
NCORES = 8
NPC = N // NCORES  # receivers per core
MASK_NEG = -50.0

# ---- tuning knobs -----------------------------------------------------------
MM_DT = "float32r"   # dtype of the big message matmul (H @ W2)
GATE_DT = "bfloat16"  # dtype of the gate matmul path (phase features + mask)
G_DT = "float32"     # dtype of the sigmoid gate tile
R9_ENGINE = "gpsimd"  # engine building the gate rhs tile
SIG_BATCH = 2        # receivers per sigmoid ACT op (PSUM banks: SIG_BATCH*2)
REPEAT = 1           # timing aid: run the compute body N times on device

_CACHE = {}


def _build_program():
    import concourse.bacc as bacc
    import concourse.mybir as mybir
    import concourse.tile as tile

    f32 = mybir.dt.float32
    mmd = getattr(mybir.dt, MM_DT)
    gmd = getattr(mybir.dt, GATE_DT)
    gdt = getattr(mybir.dt, G_DT)
    A = mybir.AluOpType
    AF = mybir.ActivationFunctionType

    nc = bacc.Bacc("TRN2", debug=False)

    def din(name, shape, dt=f32):
        return nc.declare_dram_parameter(name, list(shape), dt, isOutput=False)

    xt = din("xt", (B, D, N))          # node features, transposed
    xtr = din("xtr", (B, D, NPC))      # this core's receiver columns of xt
    p8 = din("p8", (B, 2 * O, N), gmd)      # [cos(phi); sin(phi)] all senders
    p8r = din("p8r", (B, 2 * O, NPC))  # receiver cols of p8 (fp32: scalar operand)
    mneg = din("mneg", (NPC, N), gmd)       # -50*(1-mask) rows for this core
    cinv = din("cinv", (D, NPC))       # 1/cnt_i replicated across partitions
    w1r = din("w1r", (D, D))
    w1s = din("w1s", (D, D))
    w2 = din("w2", (D, D))
    wg3 = din("wg3", (2 * O + 1, D), gmd)
    wu1x = din("wu1x", (D, D))
    wu1a = din("wu1a", (D, D))
    wu2 = din("wu2", (D, D))
    b1c = din("b1c", (D, 1))
    bgc = din("bgc", (D, 1))
    b2c = din("b2c", (D, 1))
    bu1c = din("bu1c", (D, 1))
    bu2c = din("bu2c", (D, 1))
    out = nc.declare_dram_parameter("out", [B, D, NPC], f32, isOutput=True)

    r9eng = getattr(nc, R9_ENGINE)

    with tile.TileContext(nc) as tc:
        with (
            tc.tile_pool(name="const", bufs=1) as cp,
            tc.tile_pool(name="hp", bufs=3) as hp,
            tc.tile_pool(name="r9p", bufs=4) as r9p,
            tc.tile_pool(name="gsb", bufs=2) as gsb,
            tc.tile_pool(name="mgp", bufs=2) as mgp,
            tc.tile_pool(name="sm", bufs=2) as sm,
            tc.tile_pool(name="psA", bufs=3, space="PSUM") as psA,
            tc.tile_pool(name="psG", bufs=2, space="PSUM") as psG,
            tc.tile_pool(name="psS", bufs=1, space="PSUM") as psS,
        ):
            def ct(dram, shape, dt=f32, tag=None):
                t = cp.tile(list(shape), dt, tag=tag, name=tag)
                nc.sync.dma_start(t[:], dram[:])
                return t

            w1r_t = ct(w1r, (D, D), tag="w1r")
            w1s_t = ct(w1s, (D, D), tag="w1s")
            wu1x_t = ct(wu1x, (D, D), tag="wu1x")
            wu1a_t = ct(wu1a, (D, D), tag="wu1a")
            wu2_t = ct(wu2, (D, D), tag="wu2")
            cinv_t = ct(cinv, (D, NPC), tag="cinv")
            b1c_t = ct(b1c, (D, 1), tag="b1c")
            bgc_t = ct(bgc, (D, 1), tag="bgc")
            b2c_t = ct(b2c, (D, 1), tag="b2c")
            bu1c_t = ct(bu1c, (D, 1), tag="bu1c")
            bu2c_t = ct(bu2c, (D, 1), tag="bu2c")

            # W2 must be *produced* in the matmul dtype (fp32r rounding rule)
            w2_f = ct(w2, (D, D), tag="w2f")
            if mmd != f32:
                w2_t = cp.tile([D, D], mmd, tag="w2", name="w2")
                nc.vector.tensor_copy(w2_t[:], w2_f[:])
            else:
                w2_t = w2_f
            wg3_t = ct(wg3, (2 * O + 1, D), gmd, tag="wg3")

            xt_t, p8_t, p8r_t, xtr_t = [], [], [], []
            araw = []
            for b in range(B):
                xt_b = ct(xt[b], (D, N), tag=f"xt{b}")
                xtr_b = ct(xtr[b], (D, NPC), tag=f"xtr{b}")
                p8_b = ct(p8[b], (2 * O, N), gmd, tag=f"p8{b}")
                p8r_b = ct(p8r[b], (2 * O, NPC), tag=f"p8r{b}")
                xt_t.append(xt_b)
                xtr_t.append(xtr_b)
                p8_t.append(p8_b)
                p8r_t.append(p8r_b)
                araw.append(cp.tile([D, NPC], f32, tag=f"araw{b}",
                                    name=f"araw{b}"))

            for rep in range(REPEAT):
              sendT, recvb = [], []
              for b in range(B):
                # send projection: (D, N) = W1s.T @ XT (setup: plain fp32)
                s_ps = psA.tile([D, N], f32, tag="mps", name="s_ps")
                nc.tensor.matmul(s_ps[:], w1s_t[:], xt_t[b][:],
                                 start=True, stop=True)
                s_sb = cp.tile([D, N], mmd, tag=f"send{b}", name=f"send{b}")
                nc.scalar.copy(s_sb[:], s_ps[:])
                sendT.append(s_sb)

                # receiver projection + b1: (D, NPC)
                r_ps = psS.tile([D, NPC], f32, tag="sps", name="r_ps")
                nc.tensor.matmul(r_ps[:], w1r_t[:], xtr_t[b][:],
                                 start=True, stop=True)
                r_sb = cp.tile([D, NPC], f32, tag=f"recvb{b}", name=f"recvb{b}")
                nc.scalar.add(r_sb[:], r_ps[:], b1c_t[:])
                recvb.append(r_sb)

              for b in range(B):
                for ip in range(NPC // SIG_BATCH):
                    g_ps = psG.tile([D, 512 * SIG_BATCH], f32, tag="gps", name="g_ps")
                    g_sb = gsb.tile([D, 512 * SIG_BATCH], gdt)
                    m_list = []
                    for h in range(SIG_BATCH):
                        il = ip * SIG_BATCH + h
                        # H = relu(sendT + recv_i + b1)
                        H = hp.tile([D, N], mmd)
                        nc.vector.tensor_scalar(
                            H[:], sendT[b][:], recvb[b][:, il : il + 1], 0.0,
                            op0=A.add, op1=A.max)
                        # gate rhs: rows 0-7 = P8 * P8[:, i]; row 8 = mask row
                        r9 = r9p.tile([2 * O + 1, N], gmd)
                        r9eng.tensor_scalar(
                            r9[0 : 2 * O, :], p8_t[b][:],
                            p8r_t[b][:, il : il + 1], None, op0=A.mult)
                        nc.sync.dma_start(r9[2 * O : 2 * O + 1, :],
                                          mneg[il : il + 1, :])
                        # messages (pre-bias): (D, N) in PSUM
                        m_ps = psA.tile([D, N], f32, tag="mps", name="m_ps")
                        nc.tensor.matmul(m_ps[:], w2_t[:], H[:],
                                         start=True, stop=True)
                        m_list.append(m_ps)
                        # gate linear into its half of the pair tile
                        nc.tensor.matmul(g_ps[:, h * 512 : (h + 1) * 512],
                                         wg3_t[:], r9[:],
                                         start=True, stop=True)
                    # gate = sigmoid(glin + bg), both receivers in one op
                    nc.scalar.activation(g_sb[:], g_ps[:], AF.Sigmoid,
                                         bias=bgc_t[:])
                    for h in range(SIG_BATCH):
                        il = ip * SIG_BATCH + h
                        mg = mgp.tile([D, N], f32)
                        nc.vector.scalar_tensor_tensor(
                            mg[:], m_list[h][:], b2c_t[:],
                            g_sb[:, h * 512 : (h + 1) * 512],
                            op0=A.add, op1=A.mult,
                            accum_out=araw[b][:, il : il + 1])

              for b in range(B):
                # aggregated = araw * (1/cnt)
                aggt = sm.tile([D, NPC], f32)
                nc.vector.tensor_tensor(aggt[:], araw[b][:], cinv_t[:], A.mult)
                # update net
                u_ps = psS.tile([D, NPC], f32, tag="sps", name="u_ps")
                nc.tensor.matmul(u_ps[:], wu1x_t[:], xtr_t[b][:],
                                 start=True, stop=False)
                nc.tensor.matmul(u_ps[:], wu1a_t[:], aggt[:],
                                 start=False, stop=True)
                hT = sm.tile([D, NPC], f32)
                nc.scalar.activation(hT[:], u_ps[:], AF.Relu, bias=bu1c_t[:])
                o_ps = psS.tile([D, NPC], f32, tag="sps", name="o_ps")
                nc.tensor.matmul(o_ps[:], wu2_t[:], hT[:], start=True, stop=True)
                o_sb = sm.tile([D, NPC], f32)
                nc.vector.scalar_tensor_tensor(
                    o_sb[:], o_ps[:], bu2c_t[:], xtr_t[b][:],
                    op0=A.add, op1=A.add)
                nc.sync.dma_start(out[b], o_sb[:])

    nc.compile()
    return nc


def _get_program():
    key = (MM_DT, GATE_DT, G_DT, R9_ENGINE, SIG_BATCH, REPEAT)
    if key not in _CACHE:
        _CACHE[key] = _build_program()
    return _CACHE[key]


def kernel(node_features, node_phases, adjacency,
           W1r, W1s, b1, W2, b2, Wg, bg, Wu1x, Wu1a, bu1, Wu2, bu2,
           _trace=False, _trace_kwargs=None):
    from concourse import bass_utils

    f4 = np.float32
    x = np.asarray(node_features, f4)
    ph = np.asarray(node_phases, f4)
    adj = np.asarray(adjacency)

    mask = (adj != 0)
    counts = np.maximum(mask.sum(axis=1), 1).astype(f4)          # (N,)
    cinv_full = (1.0 / counts)                                    # (N,)
    import ml_dtypes
    gnp = np.dtype(GATE_DT)
    if GATE_DT == "bfloat16":
        gnp = ml_dtypes.bfloat16
    mneg_full = (MASK_NEG * (~mask)).astype(gnp)                  # (N, N)

    xt_full = np.ascontiguousarray(x.transpose(0, 2, 1))          # (B, D, N)
    p8_f32 = np.ascontiguousarray(
        np.concatenate([np.cos(ph), np.sin(ph)], axis=2).transpose(0, 2, 1))
    p8_full = p8_f32.astype(gnp)

    common = dict(
        xt=xt_full,
        p8=p8_full,
        w1r=np.ascontiguousarray(np.asarray(W1r, f4)),
        w1s=np.ascontiguousarray(np.asarray(W1s, f4)),
        w2=np.ascontiguousarray(np.asarray(W2, f4)),
        wg3=np.ascontiguousarray(
            np.concatenate([np.asarray(Wg, f4), np.asarray(Wg, f4),
                            np.ones((1, D), f4)], axis=0)).astype(gnp),
        wu1x=np.ascontiguousarray(np.asarray(Wu1x, f4)),
        wu1a=np.ascontiguousarray(np.asarray(Wu1a, f4)),
        wu2=np.ascontiguousarray(np.asarray(Wu2, f4)),
        b1c=np.asarray(b1, f4).reshape(D, 1),
        bgc=np.asarray(bg, f4).reshape(D, 1),
        b2c=np.asarray(b2, f4).reshape(D, 1),
        bu1c=np.asarray(bu1, f4).reshape(D, 1),
        bu2c=np.asarray(bu2, f4).reshape(D, 1),
    )

    in_maps = []
    for c in range(NCORES):
        lo, hi = c * NPC, (c + 1) * NPC
        m = dict(common)
        m["xtr"] = np.ascontiguousarray(xt_full[:, :, lo:hi])
        m["p8r"] = np.ascontiguousarray(p8_f32[:, :, lo:hi])
        m["mneg"] = np.ascontiguousarray(mneg_full[lo:hi, :])
        m["cinv"] = np.ascontiguousarray(
            np.broadcast_to(cinv_full[lo:hi][None, :], (D, NPC)))
        in_maps.append(m)

    nc = _get_program()
    res = bass_utils.run_bass_kernel_spmd(
        nc, in_maps, list(range(NCORES)),
        trace=_trace, **(_trace_kwargs or {}))

    out = np.empty((B, N, D), f4)
    for c in range(NCORES):
        lo, hi = c * NPC, (c + 1) * NPC
        out[:, lo:hi, :] = res.results[c]["out"].transpose(0, 2, 1)

    kernel.last_results = res
    return out


# revision 11
# speedup vs baseline: 1.3667x; 1.3667x over previous
"""Trainium2 Bass kernel for MessagePassingWithPhase.

Reference computation (B=2, N=512, D=128, O=4):
    recv = X @ W1r ; send = X @ W1s
    hidden[b,i,j,:]  = relu(recv[b,i] + send[b,j] + b1)
    messages         = hidden @ W2 + b2
    gate             = sigmoid(cos(phi_i - phi_j) @ Wg + bg)
    agg[b,i]         = sum_j mask[i,j] * (messages * gate)[b,i,j] / cnt_i
    out              = X + (relu(X@Wu1x + agg@Wu1a + bu1) @ Wu2 + bu2)

Device mapping (8 cores, receiver axis sharded, 64 receivers/core, both batches):
  * All tiles feature-major: (D=128 partitions, node index on free axis).
  * cos(phi_i-phi_j) = cos(phi_i)cos(phi_j) + sin(phi_i)sin(phi_j): the gate
    linear becomes a K=9 matmul  Wg3.T @ r9  with
      r9[0:8,j] = P8[o,j] * P8[o,i]   (P8 = [cos(phi); sin(phi)], host-computed)
      r9[8,j]   = -50 * (1 - mask[i,j])   and Wg3 = [[Wg;Wg]; ones]
    so sigmoid(..) == gate masked to ~0 on non-neighbors: the masked mean
    becomes a plain sum over j scaled by 1/cnt_i.
  * Per receiver i:  H = relu(sendT + recvb[:,i])     (one fused tensor_scalar)
                     M = W2.T @ H            (PSUM)   (one fp32r matmul)
                     Glin = Wg3.T @ r9_i     (PSUM)   (one bf16 matmul)
                     G = sigmoid(Glin + bg)           (ACT, batched pairs)
                     agg[:,i] = sum_j (M + b2) * G    (one fused
                        scalar_tensor_tensor with accum_out reduction)
  * The gate rhs r9 is built for QUADs of 4 receivers at once: one
    broadcast-AP tensor_tensor on gpsimd + one DMA for 4 mask rows.
"""

import os
import sys
import numpy as np

for _p in ("/opt/trn_rl_repo", "/root/.axon_site/_ro/trn_rl_repo"):
    if os.path.isdir(_p) and _p not in sys.path:
        sys.path.append(_p)

B, N, D, O = 2, 512, 128, 4
NCORES = 8
NPC = N // NCORES  # receivers per core
MASK_NEG = -50.0
QUAD = 4           # receivers per gate-rhs build

# ---- tuning knobs -----------------------------------------------------------
MM_DT = "float32r"    # dtype of the big message matmul (H @ W2)
GATE_DT = "bfloat16"  # dtype of the gate matmul path (phase features + mask)
G_DT = "float32"      # dtype of the sigmoid gate tile
R9_ENGINE = "gpsimd"  # engine building the gate rhs tile
SIG_BATCH = 2         # receivers per sigmoid ACT op (PSUM banks: SIG_BATCH*2)
REPEAT = 1            # timing aid: run the compute body N times on device

_CACHE = {}


def _build_program():
    import concourse.bacc as bacc
    import concourse.mybir as mybir
    import concourse.tile as tile

    f32 = mybir.dt.float32
    mmd = getattr(mybir.dt, MM_DT)
    gmd = getattr(mybir.dt, GATE_DT)
    gdt = getattr(mybir.dt, G_DT)
    A = mybir.AluOpType
    AF = mybir.ActivationFunctionType

    nc = bacc.Bacc("TRN2", debug=False)

    def din(name, shape, dt=f32):
        return nc.declare_dram_parameter(name, list(shape), dt, isOutput=False)

    xt = din("xt", (B, D, N))          # node features, transposed
    xtr = din("xtr", (B, D, NPC))      # this core's receiver columns of xt
    p8 = din("p8", (B, 2 * O, N), gmd)  # [cos(phi); sin(phi)] all senders
    p8r = din("p8r", (B, 2 * O, NPC), gmd)  # receiver cols of p8
    mneg = din("mneg", (NPC // QUAD, QUAD * N), gmd)  # -50*(1-mask), quad rows
    cinv = din("cinv", (D, NPC))       # 1/cnt_i replicated across partitions
    w1r = din("w1r", (D, D))
    w1s = din("w1s", (D, D))
    w2 = din("w2", (D, D))
    wg3 = din("wg3", (2 * O + 1, D), gmd)
    wu1x = din("wu1x", (D, D))
    wu1a = din("wu1a", (D, D))
    wu2 = din("wu2", (D, D))
    b1c = din("b1c", (D, 1))
    bgc = din("bgc", (D, 1))
    b2c = din("b2c", (D, 1))
    bu1c = din("bu1c", (D, 1))
    bu2c = din("bu2c", (D, 1))
    out = nc.declare_dram_parameter("out", [B, D, NPC], f32, isOutput=True)

    r9eng = getattr(nc, R9_ENGINE)

    with tile.TileContext(nc) as tc:
        with (
            tc.tile_pool(name="const", bufs=1) as cp,
            tc.tile_pool(name="hp", bufs=3) as hp,
            tc.tile_pool(name="r9p", bufs=2) as r9p,
            tc.tile_pool(name="gsb", bufs=2) as gsb,
            tc.tile_pool(name="mgp", bufs=2) as mgp,
            tc.tile_pool(name="sm", bufs=2) as sm,
            tc.tile_pool(name="psA", bufs=3, space="PSUM") as psA,
            tc.tile_pool(name="psG", bufs=2, space="PSUM") as psG,
            tc.tile_pool(name="psS", bufs=1, space="PSUM") as psS,
        ):
            def ct(dram, shape, dt=f32, tag=None):
                t = cp.tile(list(shape), dt, tag=tag, name=tag)
                nc.sync.dma_start(t[:], dram[:])
                return t

            w1r_t = ct(w1r, (D, D), tag="w1r")
            w1s_t = ct(w1s, (D, D), tag="w1s")
            wu1x_t = ct(wu1x, (D, D), tag="wu1x")
            wu1a_t = ct(wu1a, (D, D), tag="wu1a")
            wu2_t = ct(wu2, (D, D), tag="wu2")
            cinv_t = ct(cinv, (D, NPC), tag="cinv")
            b1c_t = ct(b1c, (D, 1), tag="b1c")
            bgc_t = ct(bgc, (D, 1), tag="bgc")
            b2c_t = ct(b2c, (D, 1), tag="b2c")
            bu1c_t = ct(bu1c, (D, 1), tag="bu1c")
            bu2c_t = ct(bu2c, (D, 1), tag="bu2c")

            # W2 must be *produced* in the matmul dtype (fp32r rounding rule)
            w2_f = ct(w2, (D, D), tag="w2f")
            if mmd != f32:
                w2_t = cp.tile([D, D], mmd, tag="w2", name="w2")
                nc.vector.tensor_copy(w2_t[:], w2_f[:])
            else:
                w2_t = w2_f
            wg3_t = ct(wg3, (2 * O + 1, D), gmd, tag="wg3")

            xt_t, p8_t, p8r_t, xtr_t = [], [], [], []
            araw = []
            for b in range(B):
                xt_b = ct(xt[b], (D, N), tag=f"xt{b}")
                xtr_b = ct(xtr[b], (D, NPC), tag=f"xtr{b}")
                p8_b = ct(p8[b], (2 * O, N), gmd, tag=f"p8{b}")
                p8r_b = ct(p8r[b], (2 * O, NPC), gmd, tag=f"p8r{b}")
                xt_t.append(xt_b)
                xtr_t.append(xtr_b)
                p8_t.append(p8_b)
                p8r_t.append(p8r_b)
                araw.append(cp.tile([D, NPC], f32, tag=f"araw{b}",
                                    name=f"araw{b}"))

            for rep in range(REPEAT):
              sendT, recvb = [], []
              for b in range(B):
                # send projection: (D, N) = W1s.T @ XT (setup: plain fp32)
                s_ps = psA.tile([D, N], f32, tag="mps", name="s_ps")
                nc.tensor.matmul(s_ps[:], w1s_t[:], xt_t[b][:],
                                 start=True, stop=True)
                s_sb = cp.tile([D, N], mmd, tag=f"send{b}", name=f"send{b}")
                nc.scalar.copy(s_sb[:], s_ps[:])
                sendT.append(s_sb)

                # receiver projection + b1: (D, NPC)
                r_ps = psS.tile([D, NPC], f32, tag="sps", name="r_ps")
                nc.tensor.matmul(r_ps[:], w1r_t[:], xtr_t[b][:],
                                 start=True, stop=True)
                r_sb = cp.tile([D, NPC], f32, tag=f"recvb{b}", name=f"recvb{b}")
                nc.scalar.add(r_sb[:], r_ps[:], b1c_t[:])
                recvb.append(r_sb)

              for b in range(B):
                for iq in range(NPC // QUAD):
                    i0 = iq * QUAD
                    # gate rhs for 4 receivers in one op:
                    #   rows 0-7: p8[:, j] * p8[:, i]  (broadcast APs)
                    #   row 8:    4 mask rows in one DMA
                    r94 = r9p.tile([2 * O + 1, QUAD * N], gmd, name="r94")
                    r9eng.tensor_tensor(
                        r94[0 : 2 * O, :].rearrange("p (a b) -> p a b", a=QUAD),
                        p8_t[b][:].unsqueeze(1).broadcast_to((2 * O, QUAD, N)),
                        p8r_t[b][:, i0 : i0 + QUAD].unsqueeze(2)
                            .broadcast_to((2 * O, QUAD, N)),
                        A.mult)
                    nc.sync.dma_start(r94[2 * O : 2 * O + 1, :],
                                      mneg[iq : iq + 1, :])
                    for ip in range(QUAD // SIG_BATCH):
                        g_ps = psG.tile([D, 512 * SIG_BATCH], f32,
                                        tag="gps", name="g_ps")
                        g_sb = gsb.tile([D, 512 * SIG_BATCH], gdt, name="g_sb")
                        m_list = []
                        for h in range(SIG_BATCH):
                            hh = ip * SIG_BATCH + h
                            il = i0 + hh
                            # H = relu(sendT + recv_i + b1)
                            H = hp.tile([D, N], mmd, name="H")
                            nc.vector.tensor_scalar(
                                H[:], sendT[b][:],
                                recvb[b][:, il : il + 1], 0.0,
                                op0=A.add, op1=A.max)
                            # messages (pre-bias): (D, N) in PSUM
                            m_ps = psA.tile([D, N], f32, tag="mps", name="m_ps")
                            nc.tensor.matmul(m_ps[:], w2_t[:], H[:],
                                             start=True, stop=True)
                            m_list.append(m_ps)
                            # gate linear into its half of the pair tile
                            nc.tensor.matmul(
                                g_ps[:, h * 512 : (h + 1) * 512],
                                wg3_t[:], r94[:, hh * N : (hh + 1) * N],
                                start=True, stop=True)
                        # gate = sigmoid(glin + bg), SIG_BATCH recv at once
                        nc.scalar.activation(g_sb[:], g_ps[:], AF.Sigmoid,
                                             bias=bgc_t[:])
                        for h in range(SIG_BATCH):
                            il = i0 + ip * SIG_BATCH + h
                            mg = mgp.tile([D, N], f32, name="mg")
                            nc.vector.scalar_tensor_tensor(
                                mg[:], m_list[h][:], b2c_t[:],
                                g_sb[:, h * 512 : (h + 1) * 512],
                                op0=A.add, op1=A.mult,
                                accum_out=araw[b][:, il : il + 1])

              for b in range(B):
                # aggregated = araw * (1/cnt)
                aggt = sm.tile([D, NPC], f32, name="aggt")
                nc.vector.tensor_tensor(aggt[:], araw[b][:], cinv_t[:], A.mult)
                # update net
                u_ps = psS.tile([D, NPC], f32, tag="sps", name="u_ps")
                nc.tensor.matmul(u_ps[:], wu1x_t[:], xtr_t[b][:],
                                 start=True, stop=False)
                nc.tensor.matmul(u_ps[:], wu1a_t[:], aggt[:],
                                 start=False, stop=True)
                hT = sm.tile([D, NPC], f32, name="hT")
                nc.scalar.activation(hT[:], u_ps[:], AF.Relu, bias=bu1c_t[:])
                o_ps = psS.tile([D, NPC], f32, tag="sps", name="o_ps")
                nc.tensor.matmul(o_ps[:], wu2_t[:], hT[:], start=True, stop=True)
                o_sb = sm.tile([D, NPC], f32, name="o_sb")
                nc.vector.scalar_tensor_tensor(
                    o_sb[:], o_ps[:], bu2c_t[:], xtr_t[b][:],
                    op0=A.add, op1=A.add)
                nc.sync.dma_start(out[b], o_sb[:])

    nc.compile()
    return nc


def _get_program():
    key = (MM_DT, GATE_DT, G_DT, R9_ENGINE, SIG_BATCH, REPEAT, QUAD)
    if key not in _CACHE:
        _CACHE[key] = _build_program()
    return _CACHE[key]


def kernel(node_features, node_phases, adjacency,
           W1r, W1s, b1, W2, b2, Wg, bg, Wu1x, Wu1a, bu1, Wu2, bu2,
           _trace=False, _trace_kwargs=None):
    from concourse import bass_utils

    f4 = np.float32
    x = np.asarray(node_features, f4)
    ph = np.asarray(node_phases, f4)
    adj = np.asarray(adjacency)

    mask = (adj != 0)
    counts = np.maximum(mask.sum(axis=1), 1).astype(f4)          # (N,)
    cinv_full = (1.0 / counts)                                    # (N,)

    import ml_dtypes
    gnp = np.dtype(GATE_DT) if GATE_DT != "bfloat16" else ml_dtypes.bfloat16
    mneg_full = (MASK_NEG * (~mask)).astype(gnp)                  # (N, N)

    xt_full = np.ascontiguousarray(x.transpose(0, 2, 1))          # (B, D, N)
    p8_f32 = np.ascontiguousarray(
        np.concatenate([np.cos(ph), np.sin(ph)], axis=2).transpose(0, 2, 1))
    p8_full = p8_f32.astype(gnp)

    common = dict(
        xt=xt_full,
        p8=p8_full,
        w1r=np.ascontiguousarray(np.asarray(W1r, f4)),
        w1s=np.ascontiguousarray(np.asarray(W1s, f4)),
        w2=np.ascontiguousarray(np.asarray(W2, f4)),
        wg3=np.ascontiguousarray(
            np.concatenate([np.asarray(Wg, f4), np.asarray(Wg, f4),
                            np.ones((1, D), f4)], axis=0)).astype(gnp),
        wu1x=np.ascontiguousarray(np.asarray(Wu1x, f4)),
        wu1a=np.ascontiguousarray(np.asarray(Wu1a, f4)),
        wu2=np.ascontiguousarray(np.asarray(Wu2, f4)),
        b1c=np.asarray(b1, f4).reshape(D, 1),
        bgc=np.asarray(bg, f4).reshape(D, 1),
        b2c=np.asarray(b2, f4).reshape(D, 1),
        bu1c=np.asarray(bu1, f4).reshape(D, 1),
        bu2c=np.asarray(bu2, f4).reshape(D, 1),
    )

    in_maps = []
    for c in range(NCORES):
        lo, hi = c * NPC, (c + 1) * NPC
        m = dict(common)
        m["xtr"] = np.ascontiguousarray(xt_full[:, :, lo:hi])
        m["p8r"] = np.ascontiguousarray(p8_full[:, :, lo:hi])
        m["mneg"] = np.ascontiguousarray(
            mneg_full[lo:hi, :]).reshape(NPC // QUAD, QUAD * N)
        m["cinv"] = np.ascontiguousarray(
            np.broadcast_to(cinv_full[lo:hi][None, :], (D, NPC)))
        in_maps.append(m)

    nc = _get_program()
    res = bass_utils.run_bass_kernel_spmd(
        nc, in_maps, list(range(NCORES)),
        trace=_trace, **(_trace_kwargs or {}))

    out = np.empty((B, N, D), f4)
    for c in range(NCORES):
        lo, hi = c * NPC, (c + 1) * NPC
        out[:, lo:hi, :] = res.results[c]["out"].transpose(0, 2, 1)

    kernel.last_results = res
    return out


# revision 13
# speedup vs baseline: 1.4752x; 1.0794x over previous
"""Trainium2 Bass kernel for MessagePassingWithPhase.

Reference computation (B=2, N=512, D=128, O=4):
    recv = X @ W1r ; send = X @ W1s
    hidden[b,i,j,:]  = relu(recv[b,i] + send[b,j] + b1)
    messages         = hidden @ W2 + b2
    gate             = sigmoid(cos(phi_i - phi_j) @ Wg + bg)
    agg[b,i]         = sum_j mask[i,j] * (messages * gate)[b,i,j] / cnt_i
    out              = X + (relu(X@Wu1x + agg@Wu1a + bu1) @ Wu2 + bu2)

Device mapping (8 cores, receiver axis sharded, 64 receivers/core, both batches):
  * All tiles feature-major: (D=128 partitions, node index on free axis).
  * cos(phi_i-phi_j) = cos(phi_i)cos(phi_j) + sin(phi_i)sin(phi_j): the gate
    linear becomes a K=9 matmul  Wg3.T @ r9  with
      r9[0:8,j] = P8[o,j] * P8[o,i]   (P8 = [cos(phi); sin(phi)], host-computed)
      r9[8,j]   = -50 * (1 - mask[i,j])   and Wg3 = [[Wg;Wg]; ones]
    so sigmoid(..) == gate masked to ~0 on non-neighbors: the masked mean
    becomes a plain sum over j scaled by 1/cnt_i.
  * Per receiver i:  H = relu(sendT + recvb[:,i])     (one fused tensor_scalar)
                     M = W2.T @ H            (PSUM)   (one fp32r matmul)
                     Glin = Wg3.T @ r9_i     (PSUM)   (one bf16 matmul)
                     G = sigmoid(Glin + bg)           (ACT, batched pairs)
                     agg[:,i] = sum_j (M + b2) * G    (one fused
                        scalar_tensor_tensor with accum_out reduction)
  * The gate rhs r9 is built for QUADs of 4 receivers at once: one
    broadcast-AP tensor_tensor on gpsimd + one DMA for 4 mask rows.
"""

import os
import sys
import numpy as np

for _p in ("/opt/trn_rl_repo", "/root/.axon_site/_ro/trn_rl_repo"):
    if os.path.isdir(_p) and _p not in sys.path:
        sys.path.append(_p)

B, N, D, O = 2, 512, 128, 4
NCORES = 8
NPC = N // NCORES  # receivers per core
MASK_NEG = -50.0
QUAD = 4           # receivers per gate-rhs build

# ---- tuning knobs -----------------------------------------------------------
MM_DT = "float32r"    # dtype of the big message matmul (H @ W2)
GATE_DT = "bfloat16"  # dtype of the gate matmul path (phase features + mask)
G_DT = "float32"      # dtype of the sigmoid gate tile
R9_ENGINE = "gpsimd"  # engine building the gate rhs tile
SIG_BATCH = 2         # receivers per sigmoid ACT op (PSUM banks: SIG_BATCH*2)
REPEAT = 1            # timing aid: run the compute body N times on device

_CACHE = {}


def _build_program():
    import concourse.bacc as bacc
    import concourse.mybir as mybir
    import concourse.tile as tile

    f32 = mybir.dt.float32
    mmd = getattr(mybir.dt, MM_DT)
    gmd = getattr(mybir.dt, GATE_DT)
    gdt = getattr(mybir.dt, G_DT)
    A = mybir.AluOpType
    AF = mybir.ActivationFunctionType

    nc = bacc.Bacc("TRN2", debug=False)

    def din(name, shape, dt=f32):
        return nc.declare_dram_parameter(name, list(shape), dt, isOutput=False)

    xt = din("xt", (B, D, N))          # node features, transposed
    xtr = din("xtr", (B, D, NPC))      # this core's receiver columns of xt
    p8 = din("p8", (B, 2 * O, N), gmd)  # [cos(phi); sin(phi)] all senders
    p8r = din("p8r", (B, 2 * O, NPC), gmd)  # receiver cols of p8
    mneg = din("mneg", (NPC // QUAD, QUAD * N), gmd)  # -50*(1-mask), quad rows
    # all small f32 constants in one blob:
    # [w1r|w1s|w2|wu1x|wu1a|wu2|cinv|b1c|bgc|b2c|bu1c|bu2c]
    NBLOB = 6 * D + NPC + 5
    blob = din("blob", (D, NBLOB))
    wg3 = din("wg3", (2 * O + 1, D), gmd)
    out = nc.declare_dram_parameter("out", [B, D, NPC], f32, isOutput=True)

    r9eng = getattr(nc, R9_ENGINE)

    with tile.TileContext(nc) as tc:
        with (
            tc.tile_pool(name="const", bufs=1) as cp,
            tc.tile_pool(name="hp", bufs=3) as hp,
            tc.tile_pool(name="r9p", bufs=2) as r9p,
            tc.tile_pool(name="gsb", bufs=2) as gsb,
            tc.tile_pool(name="mgp", bufs=2) as mgp,
            tc.tile_pool(name="sm", bufs=2) as sm,
            tc.tile_pool(name="psA", bufs=3, space="PSUM") as psA,
            tc.tile_pool(name="psG", bufs=2, space="PSUM") as psG,
            tc.tile_pool(name="psS", bufs=1, space="PSUM") as psS,
        ):
            def ct(dram, shape, dt=f32, tag=None):
                t = cp.tile(list(shape), dt, tag=tag, name=tag)
                nc.sync.dma_start(t[:], dram[:])
                return t

            blob_t = ct(blob, (D, NBLOB), tag="blob")
            w1r_t = blob_t[:, 0 * D : 1 * D]
            w1s_t = blob_t[:, 1 * D : 2 * D]
            w2_f = blob_t[:, 2 * D : 3 * D]
            wu1x_t = blob_t[:, 3 * D : 4 * D]
            wu1a_t = blob_t[:, 4 * D : 5 * D]
            wu2_t = blob_t[:, 5 * D : 6 * D]
            cinv_t = blob_t[:, 6 * D : 6 * D + NPC]
            bofs = 6 * D + NPC
            b1c_t = blob_t[:, bofs + 0 : bofs + 1]
            bgc_t = blob_t[:, bofs + 1 : bofs + 2]
            b2c_t = blob_t[:, bofs + 2 : bofs + 3]
            bu1c_t = blob_t[:, bofs + 3 : bofs + 4]
            bu2c_t = blob_t[:, bofs + 4 : bofs + 5]

            # W2 must be *produced* in the matmul dtype (fp32r rounding rule)
            if mmd != f32:
                w2_t = cp.tile([D, D], mmd, tag="w2", name="w2")
                nc.vector.tensor_copy(w2_t, w2_f)
                w2_t = w2_t
            else:
                w2_t = w2_f
            wg3_t = ct(wg3, (2 * O + 1, D), gmd, tag="wg3")

            xt_t, p8_t, p8r_t, xtr_t = [], [], [], []
            araw = []
            for b in range(B):
                xt_b = ct(xt[b], (D, N), tag=f"xt{b}")
                xtr_b = ct(xtr[b], (D, NPC), tag=f"xtr{b}")
                p8_b = ct(p8[b], (2 * O, N), gmd, tag=f"p8{b}")
                p8r_b = ct(p8r[b], (2 * O, NPC), gmd, tag=f"p8r{b}")
                xt_t.append(xt_b)
                xtr_t.append(xtr_b)
                p8_t.append(p8_b)
                p8r_t.append(p8r_b)
                araw.append(cp.tile([D, NPC], f32, tag=f"araw{b}",
                                    name=f"araw{b}"))

            # static gate-rhs tiles: one per quad; mask row loaded once
            r94q = []
            for iq in range(NPC // QUAD):
                t = cp.tile([2 * O + 1, QUAD * N], gmd, tag=f"r94_{iq}",
                            name=f"r94_{iq}")
                nc.sync.dma_start(t[2 * O : 2 * O + 1, :], mneg[iq : iq + 1, :])
                r94q.append(t)

            for rep in range(REPEAT):
              sendT, recvb = [], []
              for b in range(B):
                # send projection: (D, N) = W1s.T @ XT (setup: plain fp32)
                s_ps = psA.tile([D, N], f32, tag="mps", name="s_ps")
                nc.tensor.matmul(s_ps[:], w1s_t, xt_t[b][:],
                                 start=True, stop=True)
                s_sb = cp.tile([D, N], mmd, tag=f"send{b}", name=f"send{b}")
                nc.scalar.copy(s_sb[:], s_ps[:])
                sendT.append(s_sb)

                # receiver projection + b1: (D, NPC)
                r_ps = psS.tile([D, NPC], f32, tag="sps", name="r_ps")
                nc.tensor.matmul(r_ps[:], w1r_t, xtr_t[b][:],
                                 start=True, stop=True)
                r_sb = cp.tile([D, NPC], f32, tag=f"recvb{b}", name=f"recvb{b}")
                nc.scalar.add(r_sb[:], r_ps[:], b1c_t)
                recvb.append(r_sb)

              for b in range(B):
                for iq in range(NPC // QUAD):
                    i0 = iq * QUAD
                    # gate rhs rows 0-7 for 4 receivers in one op
                    # (broadcast APs); mask row 8 is static.
                    r94 = r94q[iq]
                    r9eng.tensor_tensor(
                        r94[0 : 2 * O, :].rearrange("p (a b) -> p a b", a=QUAD),
                        p8_t[b][:].unsqueeze(1).broadcast_to((2 * O, QUAD, N)),
                        p8r_t[b][:, i0 : i0 + QUAD].unsqueeze(2)
                            .broadcast_to((2 * O, QUAD, N)),
                        A.mult)
                    # hidden pre-activation for the whole quad, then relu
                    pre4 = hp.tile([D, QUAD * N], f32, tag="pre4", name="pre4")
                    nc.vector.tensor_tensor(
                        pre4[:].rearrange("p (a b) -> p a b", a=QUAD),
                        sendT[b][:].unsqueeze(1).broadcast_to((D, QUAD, N)),
                        recvb[b][:, i0 : i0 + QUAD].unsqueeze(2)
                            .broadcast_to((D, QUAD, N)),
                        A.add)
                    H4 = hp.tile([D, QUAD * N], mmd, tag="H4", name="H4")
                    nc.scalar.activation(H4[:], pre4[:], AF.Relu)
                    for ip in range(QUAD // SIG_BATCH):
                        g_ps = psG.tile([D, 512 * SIG_BATCH], f32,
                                        tag="gps", name="g_ps")
                        g_sb = gsb.tile([D, 512 * SIG_BATCH], gdt, name="g_sb")
                        m_list = []
                        for h in range(SIG_BATCH):
                            hh = ip * SIG_BATCH + h
                            # messages (pre-bias): (D, N) in PSUM
                            m_ps = psA.tile([D, N], f32, tag="mps", name="m_ps")
                            nc.tensor.matmul(m_ps[:], w2_t,
                                             H4[:, hh * N : (hh + 1) * N],
                                             start=True, stop=True)
                            m_list.append(m_ps)
                            # gate linear into its half of the pair tile
                            nc.tensor.matmul(
                                g_ps[:, h * 512 : (h + 1) * 512],
                                wg3_t[:], r94[:, hh * N : (hh + 1) * N],
                                start=True, stop=True)
                        # gate = sigmoid(glin + bg), SIG_BATCH recv at once
                        nc.scalar.activation(g_sb[:], g_ps[:], AF.Sigmoid,
                                             bias=bgc_t)
                        for h in range(SIG_BATCH):
                            il = i0 + ip * SIG_BATCH + h
                            mg = mgp.tile([D, N], f32, name="mg")
                            nc.vector.scalar_tensor_tensor(
                                mg[:], m_list[h][:], b2c_t,
                                g_sb[:, h * 512 : (h + 1) * 512],
                                op0=A.add, op1=A.mult,
                                accum_out=araw[b][:, il : il + 1])

              for b in range(B):
                # aggregated = araw * (1/cnt)
                aggt = sm.tile([D, NPC], f32, name="aggt")
                nc.vector.tensor_tensor(aggt[:], araw[b][:], cinv_t, A.mult)
                # update net
                u_ps = psS.tile([D, NPC], f32, tag="sps", name="u_ps")
                nc.tensor.matmul(u_ps[:], wu1x_t, xtr_t[b][:],
                                 start=True, stop=False)
                nc.tensor.matmul(u_ps[:], wu1a_t, aggt[:],
                                 start=False, stop=True)
                hT = sm.tile([D, NPC], f32, name="hT")
                nc.scalar.activation(hT[:], u_ps[:], AF.Relu, bias=bu1c_t)
                o_ps = psS.tile([D, NPC], f32, tag="sps", name="o_ps")
                nc.tensor.matmul(o_ps[:], wu2_t, hT[:], start=True, stop=True)
                o_sb = sm.tile([D, NPC], f32, name="o_sb")
                nc.vector.scalar_tensor_tensor(
                    o_sb[:], o_ps[:], bu2c_t, xtr_t[b][:],
                    op0=A.add, op1=A.add)
                nc.sync.dma_start(out[b], o_sb[:])

    nc.compile()
    return nc


def _get_program():
    key = (MM_DT, GATE_DT, G_DT, R9_ENGINE, SIG_BATCH, REPEAT, QUAD)
    if key not in _CACHE:
        _CACHE[key] = _build_program()
    return _CACHE[key]


def kernel(node_features, node_phases, adjacency,
           W1r, W1s, b1, W2, b2, Wg, bg, Wu1x, Wu1a, bu1, Wu2, bu2,
           _trace=False, _trace_kwargs=None):
    from concourse import bass_utils

    f4 = np.float32
    x = np.asarray(node_features, f4)
    ph = np.asarray(node_phases, f4)
    adj = np.asarray(adjacency)

    mask = (adj != 0)
    counts = np.maximum(mask.sum(axis=1), 1).astype(f4)          # (N,)
    cinv_full = (1.0 / counts)                                    # (N,)

    import ml_dtypes
    gnp = np.dtype(GATE_DT) if GATE_DT != "bfloat16" else ml_dtypes.bfloat16
    mneg_full = (MASK_NEG * (~mask)).astype(gnp)                  # (N, N)

    xt_full = np.ascontiguousarray(x.transpose(0, 2, 1))          # (B, D, N)
    p8_f32 = np.ascontiguousarray(
        np.concatenate([np.cos(ph), np.sin(ph)], axis=2).transpose(0, 2, 1))
    p8_full = p8_f32.astype(gnp)

    common = dict(
        xt=xt_full,
        p8=p8_full,
        wg3=np.ascontiguousarray(
            np.concatenate([np.asarray(Wg, f4), np.asarray(Wg, f4),
                            np.ones((1, D), f4)], axis=0)).astype(gnp),
    )

    in_maps = []
    for c in range(NCORES):
        lo, hi = c * NPC, (c + 1) * NPC
        m = dict(common)
        m["xtr"] = np.ascontiguousarray(xt_full[:, :, lo:hi])
        m["p8r"] = np.ascontiguousarray(p8_full[:, :, lo:hi])
        m["mneg"] = np.ascontiguousarray(
            mneg_full[lo:hi, :]).reshape(NPC // QUAD, QUAD * N)
        cinvb = np.broadcast_to(cinv_full[lo:hi][None, :], (D, NPC))
        m["blob"] = np.ascontiguousarray(np.concatenate(
            [np.asarray(W1r, f4), np.asarray(W1s, f4), np.asarray(W2, f4),
             np.asarray(Wu1x, f4), np.asarray(Wu1a, f4), np.asarray(Wu2, f4),
             cinvb,
             np.asarray(b1, f4).reshape(D, 1), np.asarray(bg, f4).reshape(D, 1),
             np.asarray(b2, f4).reshape(D, 1), np.asarray(bu1, f4).reshape(D, 1),
             np.asarray(bu2, f4).reshape(D, 1)], axis=1))
        in_maps.append(m)

    nc = _get_program()
    res = bass_utils.run_bass_kernel_spmd(
        nc, in_maps, list(range(NCORES)),
        trace=_trace, **(_trace_kwargs or {}))

    out = np.empty((B, N, D), f4)
    for c in range(NCORES):
        lo, hi = c * NPC, (c + 1) * NPC
        out[:, lo:hi, :] = res.results[c]["out"].transpose(0, 2, 1)

    kernel.last_results = res
    return out
